# revision 1
# baseline (speedup 1.0000x reference)
"""SAN aggregation kernel for Trainium2 (Bass/Tile), 8-core data-parallel.

Problem: out[n,c,h,w] = sum_k w[n, c//8, k, h*W+w] * xpad[n, c, h+dh(k), w+dw(k)]
  x: [8, 64, 128, 128] f32, w: [8, 8, 9, 16384] f32, 3x3 window, pad 1.

Sharding: batch dim N=8 across 8 NeuronCores (1 image per core, no
cross-core communication).

Per-core layout (everything resident in SBUF):
  partitions p = hb*8 + cw   (hb: 16 row-blocks of 8 rows, cw: 8 weight chans)
  x_sb  [128, 8*10*128]: per gl, rows [hb*8-1, hb*8+9) of channel c=cw*8+gl,
        stored 128-pitch CONTIGUOUS (each (gl, partition) block loads as one
        5 KB contiguous DMA run straight from HBM).  No column padding: the
        dw=0 / dw=2 taps simply skip the output border column whose
        x-operand would be the zero pad (their contribution is zero).
        Vertical halo rows at hb=0 / hb=15 are memset to zero.
  w_sb  [128, 9*1024]:  w[cw, k, hb-rows] per partition, k-major.

Compute: all on the Vector engine (measured: GPSIMD streaming concurrently
with DVE slows DVE ~2.6x via the shared SBUF port, so offloading loses).
Per gl: 9 tensor_mul (one per tap) + 8 tensor_add accumulate, widths
127/128 by tap so no wrap-around columns are ever read.

DMA: w planes + x mains (dep-free, big) on the sync queue in consumption
order; x edge pieces and output stores on the scalar queue.  Queue FIFOs
head-of-line block on sem-waits, so dep-free loads are kept together.
"""

import sys
import os

for _p in ("/opt/trn_rl_repo", "/root/.axon_site/_ro/trn_rl_repo"):
    if _p not in sys.path and os.path.isdir(_p):
        sys.path.append(_p)

import numpy as np

import concourse.bass as bass
import concourse.bacc as bacc
import concourse.mybir as mybir
import bass_rust
from concourse.tile import TileContext
from concourse.tile_rust import add_dep_helper

F32 = mybir.dt.float32

C, H, W = 64, 128, 128
S = H * W          # 16384
CW, GL = 8, 8      # weight channels, share planes
HB = 16            # row blocks
RB = H // HB       # rows per block = 8
XROWS = RB + 2     # 10 rows incl halo
XGL = XROWS * W    # 1280 elements per gl block in x_sb
SB = RB * W        # 1024 spatial elems per partition per gl


def _ap(base, dims, extra_offset=0):
    """Copy AP `base`, replace its [step,count] dims, bump offset.

    dims[0] is the partition dim: step "P" substitutes the base AP's own
    partition stride (flat element space, = free width).
    """
    c = base.copy()
    pstep = base.ap[0][0]
    dims = [[pstep if s == "P" else s, n] for s, n in dims]
    c.ap = bass_rust.VecI64Pair(dims)
    if extra_offset:
        c.offset = c.offset + extra_offset
    return c


def build_program():
    nc = bacc.Bacc("TRN2", target_bir_lowering=False, debug=False)
    x_d = nc.dram_tensor("x", [C, S], F32, kind="ExternalInput")
    w_d = nc.dram_tensor("w", [CW, 9, S], F32, kind="ExternalInput")
    o_d = nc.dram_tensor("out", [C, S], F32, kind="ExternalOutput")

    with TileContext(nc) as tc:
        with tc.tile_pool(name="main", bufs=1) as pool, \
             tc.tile_pool(name="qtree", bufs=2) as qpool, \
             tc.tile_pool(name="os", bufs=8) as opool:
            x_sb = pool.tile([128, GL * XGL + 4], F32)  # +guards for (dw-1)/(dw+1) taps
            w_sb = pool.tile([128, 9 * SB], F32)

            # zero the vertical halo rows that have no source data:
            # r=0 at hb=0 (partitions 0..8), r=9 at hb=15 (partitions
            # 120..128); the in-range partitions are overwritten by DMA.
            nc.vector.memset(
                _ap(x_sb[:], [["P", 128], [1, 2]]), 0.0)
            nc.vector.memset(
                _ap(x_sb[:], [["P", 128], [1, 2]],
                    extra_offset=2 + GL * XGL), 0.0)
            nc.vector.memset(
                _ap(x_sb[:], [["P", 128], [XGL, GL], [1, W]],
                    extra_offset=2), 0.0)
            nc.vector.memset(
                _ap(x_sb[:], [["P", 128], [XGL, GL], [1, W]],
                    extra_offset=2 + (XROWS - 1) * W), 0.0)

            def load_w_k(k):
                nc.sync.dma_start(
                    out=_ap(w_sb[:], [["P", 128], [1, SB]],
                            extra_offset=k * SB),
                    in_=_ap(w_d.ap(), [[SB, HB], [9 * S, CW], [1, SB]],
                            extra_offset=k * S))

            def load_x_main(gl, eng):
                # partitions 8..120 (hb 1..14): rows hb*8-1 .. hb*8+9 = one
                # 1280-element contiguous run of channel c per partition.
                eng.dma_start(
                    out=_ap(x_sb[8:120], [["P", 112], [1, XGL]],
                            extra_offset=2 + gl * XGL),
                    in_=_ap(x_d.ap(), [[RB * W, HB - 2], [GL * S, CW],
                                       [1, XGL]],
                            extra_offset=gl * S + (RB - 1) * W))

            def load_x_edges(gl, eng):
                # hb=0 (partitions 0..8): rows r=1..9 = x rows 0..8
                eng.dma_start(
                    out=_ap(x_sb[0:8], [["P", 8], [1, (XROWS - 1) * W]],
                            extra_offset=2 + gl * XGL + W),
                    in_=_ap(x_d.ap(), [[GL * S, CW], [1, (XROWS - 1) * W]],
                            extra_offset=gl * S))
                # hb=15 (partitions 120..128): rows r=0..8 = x rows 119..127
                eng.dma_start(
                    out=_ap(x_sb[120:128], [["P", 8], [1, (XROWS - 1) * W]],
                            extra_offset=2 + gl * XGL),
                    in_=_ap(x_d.ap(), [[GL * S, CW], [1, (XROWS - 1) * W]],
                            extra_offset=gl * S + (H - XROWS + 1) * W))

            # Issue order: sync = w planes + x0/x1 only (so gl0/gl1's
            # inputs and all w land without queuing behind later loads);
            # scalar = edge pieces, then x2..x7 mains, then (later) the
            # output stores.  Splitting the mains keeps each queue's
            # 9-semaphore recycle window shallow.
            # Queue/ring discipline (all measured):
            #  - scalar/ACT issues no DMAs until ~10us (framework preamble)
            #    -> gl0's working set must ride sync;
            #  - narrow 8-partition edge DMAs use only 1-2 of the 16 rings
            #    and their packets queue FIFO behind any bulk already
            #    enqueued -> issue edges BEFORE the bulk on each queue.
            load_x_edges(0, nc.sync)
            load_w_k(0)
            load_w_k(1)
            load_x_main(0, nc.sync)
            for k in range(2, 9):
                load_w_k(k)
            for gl in range(1, GL):
                load_x_edges(gl, nc.scalar)
            for gl in range(1, GL):
                load_x_main(gl, nc.scalar)

            # ---- compute (all DVE) ----
            # tap (dh, dw): out[h', w] += w_k[h', w] * x[r=h'+dh, w+dw-1];
            # dw=0 skips output col 0, dw=2 skips output col W-1 (their
            # x operand is the zero pad).
            def out_dma(gl, src):
                # sync queue is idle once the w/x0 loads drain (~20us);
                # keeping stores off scalar speeds x2..7 delivery.
                nc.sync.dma_start(
                    out=_ap(o_d.ap(), [[RB * W, HB], [GL * S, CW], [1, SB]],
                            extra_offset=gl * S),
                    in_=src)

            prev_last = None   # pin gl order: the static scheduler
            # otherwise reorders chains by its (wrong) DMA timing model,
            # head-of-line blocking the DVE queue on late inputs.

            # All gls use mult+add chains: 2 DVE ops per arriving w
            # plane matches the DMA delivery rate during the ramp (wide
            # tree reductions and full-width taps gated on w-border
            # memsets both measured slower end-to-end).
            # tap (dh, dw): out[h', w] += w_k[h', w] * x[r=h'+dh, w+dw-1];
            # dw=0 skips output col 0, dw=2 skips col W-1 (their x operand
            # is the zero pad, so the contribution is zero).
            for gl in range(GL):
                acc_t = opool.tile([128, SB], F32, tag="o", name="acc_t")
                first = True
                for k in range(9):
                    dh, dw = divmod(k, 3)
                    w0 = 1 if dw == 0 else 0          # output start col
                    cnt = W - 1 if dw != 1 else W     # output width
                    xoff = 2 + gl * XGL + dh * W + (1 if dw == 2 else 0)
                    xvw = _ap(x_sb[:], [["P", 128], [W, RB], [1, cnt]],
                              extra_offset=xoff)
                    wvw = _ap(w_sb[:], [["P", 128], [W, RB], [1, cnt]],
                              extra_offset=k * SB + w0)
                    accw = _ap(acc_t[:], [["P", 128], [W, RB], [1, cnt]],
                               extra_offset=w0)
                    if first:
                        # k=0 (a dw=0 tap) leaves output col 0 unwritten:
                        # zero the two border columns once.
                        nc.vector.memset(
                            _ap(acc_t[:], [["P", 128], [W, RB],
                                           [W - 1, 2]]), 0.0)
                        m = nc.vector.tensor_mul(out=accw, in0=xvw,
                                                 in1=wvw)
                        if prev_last is not None:
                            add_dep_helper(m.ins, prev_last.ins, sync=False,
                                           reason="gl chain order")
                        first = False
                    else:
                        tmp = qpool.tile([128, SB], F32, tag="tmp",
                                         name="tmp")
                        t = _ap(tmp[:], [["P", 128], [W, RB], [1, cnt]],
                                extra_offset=w0)
                        nc.vector.tensor_mul(out=t, in0=xvw, in1=wvw)
                        prev_last = nc.vector.tensor_add(out=accw,
                                                         in0=accw, in1=t)
                out_dma(gl, _ap(acc_t[:], [["P", 128], [1, SB]]))

    nc.compile()
    return nc


_NC_CACHE = None


def _get_nc():
    global _NC_CACHE
    if _NC_CACHE is None:
        _NC_CACHE = build_program()
    return _NC_CACHE


def kernel(input, weight):
    """input: [8,64,128,128] f32, weight: [8,8,9,16384] f32 ->
    [8,64,128,128] f32."""
    from concourse.bass_utils import run_bass_kernel_spmd

    x = np.ascontiguousarray(np.asarray(input, dtype=np.float32))
    w = np.ascontiguousarray(np.asarray(weight, dtype=np.float32))
    N = x.shape[0]
    nc = _get_nc()
    in_maps = [{"x": x[i].reshape(C, S), "w": w[i].reshape(CW, 9, S)}
               for i in range(N)]
    res = run_bass_kernel_spmd(nc, in_maps, core_ids=list(range(N)))
    out = np.stack([res.results[i]["out"].reshape(C, H, W) for i in range(N)])
    return out



# revision 5
# speedup vs baseline: 1.2882x; 1.2882x over previous
"""SAN aggregation kernel for Trainium2 (Bass/Tile), 8-core data-parallel.

Problem: out[n,c,h,w] = sum_k w[n, c//8, k, h*W+w] * xpad[n, c, h+dh(k), w+dw(k)]
  x: [8, 64, 128, 128] f32, w: [8, 8, 9, 16384] f32, 3x3 window, pad 1.

Sharding: batch dim N=8 across 8 NeuronCores (1 image per core, no
cross-core communication).

Per-core layout (everything resident in SBUF):
  partitions p = hb*8 + cw   (hb: 16 row-blocks of 8 rows, cw: 8 weight chans)
  x_sb  [128, 8*10*128]: per gl, rows [hb*8-1, hb*8+9) of channel c=cw*8+gl,
        stored 128-pitch CONTIGUOUS (each (gl, partition) block loads as one
        5 KB contiguous DMA run straight from HBM).  No column padding: the
        dw=0 / dw=2 taps simply skip the output border column whose
        x-operand would be the zero pad (their contribution is zero).
        Vertical halo rows at hb=0 / hb=15 are memset to zero.
  w_sb  [128, 9*1024]:  w[cw, k, hb-rows] per partition, k-major.

Compute: all on the Vector engine (measured: GPSIMD streaming concurrently
with DVE slows DVE ~2.6x via the shared SBUF port, so offloading loses).
Per gl: 9 tensor_mul (one per tap) + 8 tensor_add accumulate, widths
127/128 by tap so no wrap-around columns are ever read.

DMA: w planes + x mains (dep-free, big) on the sync queue in consumption
order; x edge pieces and output stores on the scalar queue.  Queue FIFOs
head-of-line block on sem-waits, so dep-free loads are kept together.
"""

import sys
import os

for _p in ("/opt/trn_rl_repo", "/root/.axon_site/_ro/trn_rl_repo"):
    if _p not in sys.path and os.path.isdir(_p):
        sys.path.append(_p)

import numpy as np

import concourse.bass as bass
import concourse.bacc as bacc
import concourse.mybir as mybir
import bass_rust
from concourse.tile import TileContext
from concourse.tile_rust import add_dep_helper

F32 = mybir.dt.float32
F16 = mybir.dt.float16

C, H, W = 64, 128, 128
S = H * W          # 16384
CW, GL = 8, 8      # weight channels, share planes
HB = 16            # row blocks
RB = H // HB       # rows per block = 8
XROWS = RB + 2     # 10 rows incl halo
XGL = XROWS * W    # 1280 elements per gl block in x_sb
SB = RB * W        # 1024 spatial elems per partition per gl


def _ap(base, dims, extra_offset=0):
    """Copy AP `base`, replace its [step,count] dims, bump offset.

    dims[0] is the partition dim: step "P" substitutes the base AP's own
    partition stride (flat element space, = free width).
    """
    c = base.copy()
    pstep = base.ap[0][0]
    dims = [[pstep if s == "P" else s, n] for s, n in dims]
    c.ap = bass_rust.VecI64Pair(dims)
    if extra_offset:
        c.offset = c.offset + extra_offset
    return c


def build_program():
    nc = bacc.Bacc("TRN2", target_bir_lowering=False, debug=False)
    x_d = nc.dram_tensor("x", [C, S], F32, kind="ExternalInput")
    w_d = nc.dram_tensor("w", [CW, 9, S], F32, kind="ExternalInput")
    o_d = nc.dram_tensor("out", [C, S], F32, kind="ExternalOutput")

    with TileContext(nc) as tc:
        with tc.tile_pool(name="main", bufs=1) as pool, \
             tc.tile_pool(name="qtree", bufs=2) as qpool, \
             tc.tile_pool(name="os", bufs=8) as opool, \
             tc.tile_pool(name="o32", bufs=4) as o32pool:
            x_sb = pool.tile([128, GL * XGL + 4], F32)  # +guards for (dw-1)/(dw+1) taps
            w_sb = pool.tile([128, 9 * SB], F32)
            # fp16 copies (cast on the idle ACT engine): DVE 16-bit packed
            # ops run in the 2x/4x perf mode, halving/quartering the
            # per-element cost vs f32.
            x16 = pool.tile([128, GL * XGL + 4], F16)
            w16 = pool.tile([128, 9 * SB], F16)

            # zero the vertical halo rows that have no source data:
            # r=0 at hb=0 (partitions 0..8), r=9 at hb=15 (partitions
            # 120..128); the in-range partitions are overwritten by DMA.
            nc.vector.memset(
                _ap(x_sb[:], [["P", 128], [1, 2]]), 0.0)
            nc.vector.memset(
                _ap(x_sb[:], [["P", 128], [1, 2]],
                    extra_offset=2 + GL * XGL), 0.0)
            nc.vector.memset(
                _ap(x_sb[:], [["P", 128], [XGL, GL], [1, W]],
                    extra_offset=2), 0.0)
            nc.vector.memset(
                _ap(x_sb[:], [["P", 128], [XGL, GL], [1, W]],
                    extra_offset=2 + (XROWS - 1) * W), 0.0)

            def load_w_k(k):
                nc.sync.dma_start(
                    out=_ap(w_sb[:], [["P", 128], [1, SB]],
                            extra_offset=k * SB),
                    in_=_ap(w_d.ap(), [[SB, HB], [9 * S, CW], [1, SB]],
                            extra_offset=k * S))

            def load_x_main(gl, eng):
                # partitions 8..120 (hb 1..14): rows hb*8-1 .. hb*8+9 = one
                # 1280-element contiguous run of channel c per partition.
                eng.dma_start(
                    out=_ap(x_sb[8:120], [["P", 112], [1, XGL]],
                            extra_offset=2 + gl * XGL),
                    in_=_ap(x_d.ap(), [[RB * W, HB - 2], [GL * S, CW],
                                       [1, XGL]],
                            extra_offset=gl * S + (RB - 1) * W))

            def load_x_edges(gl, eng):
                # hb=0 (partitions 0..8): rows r=1..9 = x rows 0..8
                eng.dma_start(
                    out=_ap(x_sb[0:8], [["P", 8], [1, (XROWS - 1) * W]],
                            extra_offset=2 + gl * XGL + W),
                    in_=_ap(x_d.ap(), [[GL * S, CW], [1, (XROWS - 1) * W]],
                            extra_offset=gl * S))
                # hb=15 (partitions 120..128): rows r=0..8 = x rows 119..127
                eng.dma_start(
                    out=_ap(x_sb[120:128], [["P", 8], [1, (XROWS - 1) * W]],
                            extra_offset=2 + gl * XGL),
                    in_=_ap(x_d.ap(), [[GL * S, CW], [1, (XROWS - 1) * W]],
                            extra_offset=gl * S + (H - XROWS + 1) * W))

            # Issue order: ALL loads ride the sync queue in consumption
            # order (gl0's working set first).  The scalar/ACT sequencer
            # is reserved for the f32->fp16 cast stream: a dma_start on
            # ACT costs ~667ns of sequencer time each, which would delay
            # the casts the DVE consumes.
            load_x_edges(0, nc.sync)
            load_w_k(0)
            load_x_main(0, nc.sync)
            load_w_k(1)
            load_w_k(2)
            load_w_k(3)
            load_x_edges(1, nc.sync)
            load_x_main(1, nc.sync)
            for k in range(4, 9):
                load_w_k(k)
            for gl in range(2, GL):
                load_x_edges(gl, nc.sync)
                load_x_main(gl, nc.sync)

            # ---- f32 -> fp16 casts (ACT engine, otherwise idle) ----
            def cast_w(k):
                nc.scalar.copy(
                    out=_ap(w16[:], [["P", 128], [1, SB]],
                            extra_offset=k * SB),
                    in_=_ap(w_sb[:], [["P", 128], [1, SB]],
                            extra_offset=k * SB))

            def cast_x(gl):
                nc.scalar.copy(
                    out=_ap(x16[:], [["P", 128], [1, XGL]],
                            extra_offset=2 + gl * XGL),
                    in_=_ap(x_sb[:], [["P", 128], [1, XGL]],
                            extra_offset=2 + gl * XGL))

            cast_w(0)
            cast_x(0)
            for k in range(1, 9):
                cast_w(k)
            for gl in range(1, GL):
                cast_x(gl)

            # ---- compute (all DVE) ----
            # tap (dh, dw): out[h', w] += w_k[h', w] * x[r=h'+dh, w+dw-1];
            # dw=0 skips output col 0, dw=2 skips output col W-1 (their
            # x operand is the zero pad).
            def out_dma(gl, src):
                # sync queue is idle once the w/x0 loads drain (~20us);
                # keeping stores off scalar speeds x2..7 delivery.
                nc.sync.dma_start(
                    out=_ap(o_d.ap(), [[RB * W, HB], [GL * S, CW], [1, SB]],
                            extra_offset=gl * S),
                    in_=src)

            prev_last = None   # pin gl order: the static scheduler
            # otherwise reorders chains by its (wrong) DMA timing model,
            # head-of-line blocking the DVE queue on late inputs.

            # All gls use mult+add chains: 2 DVE ops per arriving w
            # plane matches the DMA delivery rate during the ramp (wide
            # tree reductions and full-width taps gated on w-border
            # memsets both measured slower end-to-end).
            # tap (dh, dw): out[h', w] += w_k[h', w] * x[r=h'+dh, w+dw-1];
            # dw=0 skips output col 0, dw=2 skips col W-1 (their x operand
            # is the zero pad, so the contribution is zero).
            for gl in range(GL):
                acc_t = opool.tile([128, SB], F16, tag="o", name="acc_t")
                first = True
                for k in range(9):
                    dh, dw = divmod(k, 3)
                    w0 = 1 if dw == 0 else 0          # output start col
                    cnt = W - 1 if dw != 1 else W     # output width
                    xoff = 2 + gl * XGL + dh * W + (1 if dw == 2 else 0)
                    xvw = _ap(x16[:], [["P", 128], [W, RB], [1, cnt]],
                              extra_offset=xoff)
                    wvw = _ap(w16[:], [["P", 128], [W, RB], [1, cnt]],
                              extra_offset=k * SB + w0)
                    accw = _ap(acc_t[:], [["P", 128], [W, RB], [1, cnt]],
                               extra_offset=w0)
                    if first:
                        # k=0 (a dw=0 tap) leaves output col 0 unwritten:
                        # zero the two border columns once.
                        nc.vector.memset(
                            _ap(acc_t[:], [["P", 128], [W, RB],
                                           [W - 1, 2]]), 0.0)
                        m = nc.vector.tensor_mul(out=accw, in0=xvw,
                                                 in1=wvw)
                        if prev_last is not None:
                            add_dep_helper(m.ins, prev_last.ins, sync=False,
                                           reason="gl chain order")
                        first = False
                    else:
                        tmp = qpool.tile([128, SB], F16, tag="tmp",
                                         name="tmp")
                        t = _ap(tmp[:], [["P", 128], [W, RB], [1, cnt]],
                                extra_offset=w0)
                        nc.vector.tensor_mul(out=t, in0=xvw, in1=wvw)
                        prev_last = nc.vector.tensor_add(out=accw,
                                                         in0=accw, in1=t)
                # fp16 acc -> f32 staging on ACT, then store
                o32 = o32pool.tile([128, SB], F32, tag="o32", name="o32")
                nc.scalar.copy(out=_ap(o32[:], [["P", 128], [1, SB]]),
                               in_=_ap(acc_t[:], [["P", 128], [1, SB]]))
                out_dma(gl, _ap(o32[:], [["P", 128], [1, SB]]))

    nc.compile()
    return nc


_NC_CACHE = None


def _get_nc():
    global _NC_CACHE
    if _NC_CACHE is None:
        _NC_CACHE = build_program()
    return _NC_CACHE


def kernel(input, weight):
    """input: [8,64,128,128] f32, weight: [8,8,9,16384] f32 ->
    [8,64,128,128] f32."""
    from concourse.bass_utils import run_bass_kernel_spmd

    x = np.ascontiguousarray(np.asarray(input, dtype=np.float32))
    w = np.ascontiguousarray(np.asarray(weight, dtype=np.float32))
    N = x.shape[0]
    nc = _get_nc()
    in_maps = [{"x": x[i].reshape(C, S), "w": w[i].reshape(CW, 9, S)}
               for i in range(N)]
    res = run_bass_kernel_spmd(nc, in_maps, core_ids=list(range(N)))
    out = np.stack([res.results[i]["out"].reshape(C, H, W) for i in range(N)])
    return out



# revision 7
# speedup vs baseline: 1.6141x; 1.2530x over previous
"""SAN aggregation kernel for Trainium2 (Bass/Tile), 8-core data-parallel.

Problem: out[n,c,h,w] = sum_k w[n, c//8, k, h*W+w] * xpad[n, c, h+dh(k), w+dw(k)]
  x: [8, 64, 128, 128] f32, w: [8, 8, 9, 16384] f32, 3x3 window, pad 1.

Sharding: batch dim N=8 across 8 NeuronCores (1 image per core, no
cross-core communication).

Per-core layout (everything resident in SBUF):
  partitions p = hb*8 + cw   (hb: 16 row-blocks of 8 rows, cw: 8 weight chans)
  x_sb  [128, 8*10*128]: per gl, rows [hb*8-1, hb*8+9) of channel c=cw*8+gl,
        stored 128-pitch CONTIGUOUS (each (gl, partition) block loads as one
        5 KB contiguous DMA run straight from HBM).  No column padding: the
        dw=0 / dw=2 taps simply skip the output border column whose
        x-operand would be the zero pad (their contribution is zero).
        Vertical halo rows at hb=0 / hb=15 are memset to zero.
  w_sb  [128, 9*1024]:  w[cw, k, hb-rows] per partition, k-major.

Compute: all on the Vector engine (measured: GPSIMD streaming concurrently
with DVE slows DVE ~2.6x via the shared SBUF port, so offloading loses).
Per gl: 9 tensor_mul (one per tap) + 8 tensor_add accumulate, widths
127/128 by tap so no wrap-around columns are ever read.

DMA: w planes + x mains (dep-free, big) on the sync queue in consumption
order; x edge pieces and output stores on the scalar queue.  Queue FIFOs
head-of-line block on sem-waits, so dep-free loads are kept together.
"""

import sys
import os

for _p in ("/opt/trn_rl_repo", "/root/.axon_site/_ro/trn_rl_repo"):
    if _p not in sys.path and os.path.isdir(_p):
        sys.path.append(_p)

import numpy as np

import concourse.bass as bass
import concourse.bacc as bacc
import concourse.mybir as mybir
import bass_rust
from concourse.tile import TileContext
from concourse.tile_rust import add_dep_helper

F32 = mybir.dt.float32
F16 = mybir.dt.float16

C, H, W = 64, 128, 128
S = H * W          # 16384
CW, GL = 8, 8      # weight channels, share planes
HB = 16            # row blocks
RB = H // HB       # rows per block = 8
XROWS = RB + 2     # 10 rows incl halo
XGL = XROWS * W    # 1280 elements per gl block in x_sb
SB = RB * W        # 1024 spatial elems per partition per gl


def _ap(base, dims, extra_offset=0):
    """Copy AP `base`, replace its [step,count] dims, bump offset.

    dims[0] is the partition dim: step "P" substitutes the base AP's own
    partition stride (flat element space, = free width).
    """
    c = base.copy()
    pstep = base.ap[0][0]
    dims = [[pstep if s == "P" else s, n] for s, n in dims]
    c.ap = bass_rust.VecI64Pair(dims)
    if extra_offset:
        c.offset = c.offset + extra_offset
    return c


def build_program():
    nc = bacc.Bacc("TRN2", target_bir_lowering=False, debug=False)
    x_d = nc.dram_tensor("x", [C, S], F32, kind="ExternalInput")
    w_d = nc.dram_tensor("w", [CW, 9, S], F32, kind="ExternalInput")
    o_d = nc.dram_tensor("out", [C, S], F32, kind="ExternalOutput")

    with TileContext(nc) as tc:
        with tc.tile_pool(name="main", bufs=1) as pool, \
             tc.tile_pool(name="qtree", bufs=2) as qpool, \
             tc.tile_pool(name="os", bufs=8) as opool, \
             tc.tile_pool(name="o32", bufs=4) as o32pool:
            x_sb = pool.tile([128, GL * XGL + 4], F32)  # +guards for (dw-1)/(dw+1) taps
            w_sb = pool.tile([128, 9 * SB], F32)
            # fp16 copies (cast on the idle ACT engine): DVE 16-bit packed
            # ops run in the 2x/4x perf mode, halving/quartering the
            # per-element cost vs f32.
            x16 = pool.tile([128, GL * XGL + 4], F16)
            w16 = pool.tile([128, 9 * SB], F16)

            # zero the vertical halo rows that have no source data:
            # r=0 at hb=0 (partitions 0..8), r=9 at hb=15 (partitions
            # 120..128); the in-range partitions are overwritten by DMA.
            nc.vector.memset(
                _ap(x_sb[:], [["P", 128], [1, 2]]), 0.0)
            nc.vector.memset(
                _ap(x_sb[:], [["P", 128], [1, 2]],
                    extra_offset=2 + GL * XGL), 0.0)
            nc.vector.memset(
                _ap(x_sb[:], [["P", 128], [XGL, GL], [1, W]],
                    extra_offset=2), 0.0)
            nc.vector.memset(
                _ap(x_sb[:], [["P", 128], [XGL, GL], [1, W]],
                    extra_offset=2 + (XROWS - 1) * W), 0.0)

            def load_w_k(k):
                nc.sync.dma_start(
                    out=_ap(w_sb[:], [["P", 128], [1, SB]],
                            extra_offset=k * SB),
                    in_=_ap(w_d.ap(), [[SB, HB], [9 * S, CW], [1, SB]],
                            extra_offset=k * S))

            def load_x_main(gl, eng):
                # partitions 8..120 (hb 1..14): rows hb*8-1 .. hb*8+9 = one
                # 1280-element contiguous run of channel c per partition.
                eng.dma_start(
                    out=_ap(x_sb[8:120], [["P", 112], [1, XGL]],
                            extra_offset=2 + gl * XGL),
                    in_=_ap(x_d.ap(), [[RB * W, HB - 2], [GL * S, CW],
                                       [1, XGL]],
                            extra_offset=gl * S + (RB - 1) * W))

            def load_x_edges(gl, eng):
                # hb=0 (partitions 0..8): rows r=1..9 = x rows 0..8
                eng.dma_start(
                    out=_ap(x_sb[0:8], [["P", 8], [1, (XROWS - 1) * W]],
                            extra_offset=2 + gl * XGL + W),
                    in_=_ap(x_d.ap(), [[GL * S, CW], [1, (XROWS - 1) * W]],
                            extra_offset=gl * S))
                # hb=15 (partitions 120..128): rows r=0..8 = x rows 119..127
                eng.dma_start(
                    out=_ap(x_sb[120:128], [["P", 8], [1, (XROWS - 1) * W]],
                            extra_offset=2 + gl * XGL),
                    in_=_ap(x_d.ap(), [[GL * S, CW], [1, (XROWS - 1) * W]],
                            extra_offset=gl * S + (H - XROWS + 1) * W))

            # Issue order: ALL loads ride the sync queue in consumption
            # order (gl0's working set first).  The scalar/ACT sequencer
            # is reserved for the f32->fp16 cast stream: a dma_start on
            # ACT costs ~667ns of sequencer time each, which would delay
            # the casts the DVE consumes.
            load_x_edges(0, nc.sync)
            load_w_k(0)
            load_x_main(0, nc.sync)
            load_w_k(1)
            load_w_k(2)
            load_w_k(3)
            load_x_edges(1, nc.sync)
            load_x_main(1, nc.sync)
            for k in range(4, 9):
                load_w_k(k)
            for gl in range(2, GL):
                load_x_edges(gl, nc.sync)
                load_x_main(gl, nc.sync)

            # ---- f32 -> fp16 casts (ACT engine, otherwise idle) ----
            def cast_w(k):
                nc.scalar.copy(
                    out=_ap(w16[:], [["P", 128], [1, SB]],
                            extra_offset=k * SB),
                    in_=_ap(w_sb[:], [["P", 128], [1, SB]],
                            extra_offset=k * SB))

            def cast_x(gl):
                nc.scalar.copy(
                    out=_ap(x16[:], [["P", 128], [1, XGL]],
                            extra_offset=2 + gl * XGL),
                    in_=_ap(x_sb[:], [["P", 128], [1, XGL]],
                            extra_offset=2 + gl * XGL))

            cast_w(0)
            cast_x(0)
            for k in range(1, 9):
                cast_w(k)
            for gl in range(1, GL):
                cast_x(gl)

            # ---- compute (all DVE) ----
            # tap (dh, dw): out[h', w] += w_k[h', w] * x[r=h'+dh, w+dw-1];
            # dw=0 skips output col 0, dw=2 skips output col W-1 (their
            # x operand is the zero pad).
            def out_dma(gl, src):
                # sync queue is idle once the w/x0 loads drain (~20us);
                # keeping stores off scalar speeds x2..7 delivery.
                nc.sync.dma_start(
                    out=_ap(o_d.ap(), [[RB * W, HB], [GL * S, CW], [1, SB]],
                            extra_offset=gl * S),
                    in_=src)

            prev_last = None   # pin gl order: the static scheduler
            # otherwise reorders chains by its (wrong) DMA timing model,
            # head-of-line blocking the DVE queue on late inputs.

            # All gls use mult+add chains: 2 DVE ops per arriving w
            # plane matches the DMA delivery rate during the ramp (wide
            # tree reductions and full-width taps gated on w-border
            # memsets both measured slower end-to-end).
            # tap (dh, dw): out[h', w] += w_k[h', w] * x[r=h'+dh, w+dw-1];
            # dw=0 skips output col 0, dw=2 skips col W-1 (their x operand
            # is the zero pad, so the contribution is zero).
            # Process gl PAIRS per DVE op (4D APs, w broadcast across the
            # pair via stride 0): halves the DVE instruction count, so the
            # ~300ns/op fixed overhead is amortized over 2x the elements.
            NG = 2                     # gls per DVE op
            for g0 in range(0, GL, NG):
                acc_t = opool.tile([128, NG * SB], F16, tag="o",
                                   name="acc_t")
                o32 = o32pool.tile([128, NG * SB], F32, tag="o32",
                                   name="o32")
                last_pair = g0 + NG >= GL
                first = True
                for k in range(9):
                    dh, dw = divmod(k, 3)
                    w0 = 1 if dw == 0 else 0          # output start col
                    cnt = W - 1 if dw != 1 else W     # output width
                    xoff = 2 + g0 * XGL + dh * W + (1 if dw == 2 else 0)
                    xvw = _ap(x16[:], [["P", 128], [XGL, NG], [W, RB],
                                       [1, cnt]],
                              extra_offset=xoff)
                    wvw = _ap(w16[:], [["P", 128], [0, NG], [W, RB],
                                       [1, cnt]],
                              extra_offset=k * SB + w0)
                    accw = _ap(acc_t[:], [["P", 128], [SB, NG], [W, RB],
                                          [1, cnt]],
                               extra_offset=w0)
                    if first:
                        # k=0 (a dw=0 tap) leaves output col 0 unwritten:
                        # zero the two border columns once.
                        nc.vector.memset(
                            _ap(acc_t[:], [["P", 128], [SB, NG], [W, RB],
                                           [W - 1, 2]]), 0.0)
                        m = nc.vector.tensor_mul(out=accw, in0=xvw,
                                                 in1=wvw)
                        if prev_last is not None:
                            add_dep_helper(m.ins, prev_last.ins, sync=False,
                                           reason="gl chain order")
                        first = False
                    else:
                        tmp = qpool.tile([128, NG * SB], F16, tag="tmp",
                                         name="tmp")
                        t = _ap(tmp[:], [["P", 128], [SB, NG], [W, RB],
                                         [1, cnt]],
                                extra_offset=w0)
                        nc.vector.tensor_mul(out=t, in0=xvw, in1=wvw)
                        if last_pair and k == 8:
                            # final add writes f32 directly: runs 1x but
                            # skips the serial ACT out-cast on the tail.
                            prev_last = nc.vector.tensor_add(
                                out=_ap(o32[:], [["P", 128], [SB, NG],
                                                 [W, RB], [1, cnt]],
                                        extra_offset=w0),
                                in0=accw, in1=t)
                            # col W-1 of o32: k=8 (dw=2) never writes it
                            # and its contribution there is zero, so it
                            # comes straight from acc_t (k<=7 sum).
                            nc.vector.tensor_scalar_add(
                                out=_ap(o32[:], [["P", 128], [SB, NG],
                                                 [W, RB]],
                                        extra_offset=W - 1),
                                in0=_ap(acc_t[:], [["P", 128], [SB, NG],
                                                   [W, RB]],
                                        extra_offset=W - 1),
                                scalar1=0.0)
                        else:
                            prev_last = nc.vector.tensor_add(out=accw,
                                                             in0=accw,
                                                             in1=t)
                if not last_pair:
                    # fp16 acc -> f32 staging on ACT, then store
                    nc.scalar.copy(
                        out=_ap(o32[:], [["P", 128], [1, NG * SB]]),
                        in_=_ap(acc_t[:], [["P", 128], [1, NG * SB]]))
                for g in range(NG):
                    out_dma(g0 + g, _ap(o32[:], [["P", 128], [1, SB]],
                                        extra_offset=g * SB))

    nc.compile()
    return nc


_NC_CACHE = None


def _get_nc():
    global _NC_CACHE
    if _NC_CACHE is None:
        _NC_CACHE = build_program()
    return _NC_CACHE


def kernel(input, weight):
    """input: [8,64,128,128] f32, weight: [8,8,9,16384] f32 ->
    [8,64,128,128] f32."""
    from concourse.bass_utils import run_bass_kernel_spmd

    x = np.ascontiguousarray(np.asarray(input, dtype=np.float32))
    w = np.ascontiguousarray(np.asarray(weight, dtype=np.float32))
    N = x.shape[0]
    nc = _get_nc()
    in_maps = [{"x": x[i].reshape(C, S), "w": w[i].reshape(CW, 9, S)}
               for i in range(N)]
    res = run_bass_kernel_spmd(nc, in_maps, core_ids=list(range(N)))
    out = np.stack([res.results[i]["out"].reshape(C, H, W) for i in range(N)])
    return out



# revision 12
# speedup vs baseline: 1.6512x; 1.0230x over previous
"""SAN aggregation kernel for Trainium2 (Bass/Tile), 8-core data-parallel.

Problem: out[n,c,h,w] = sum_k w[n, c//8, k, h*W+w] * xpad[n, c, h+dh(k), w+dw(k)]
  x: [8, 64, 128, 128] f32, w: [8, 8, 9, 16384] f32, 3x3 window, pad 1.

Sharding: batch dim N=8 across 8 NeuronCores (1 image per core, no
cross-core communication).

Per-core layout (everything resident in SBUF):
  partitions p = hb*8 + cw   (hb: 16 row-blocks of 8 rows, cw: 8 weight chans)
  x_sb  [128, 8*10*128]: per gl, rows [hb*8-1, hb*8+9) of channel c=cw*8+gl,
        stored 128-pitch CONTIGUOUS (each (gl, partition) block loads as one
        5 KB contiguous DMA run straight from HBM).  No column padding: the
        dw=0 / dw=2 taps simply skip the output border column whose
        x-operand would be the zero pad (their contribution is zero).
        Vertical halo rows at hb=0 / hb=15 are memset to zero.
  w_sb  [128, 9*1024]:  w[cw, k, hb-rows] per partition, k-major.

Compute: all on the Vector engine (measured: GPSIMD streaming concurrently
with DVE slows DVE ~2.6x via the shared SBUF port, so offloading loses).
Per gl: 9 tensor_mul (one per tap) + 8 tensor_add accumulate, widths
127/128 by tap so no wrap-around columns are ever read.

DMA: w planes + x mains (dep-free, big) on the sync queue in consumption
order; x edge pieces and output stores on the scalar queue.  Queue FIFOs
head-of-line block on sem-waits, so dep-free loads are kept together.
"""

import sys
import os

for _p in ("/opt/trn_rl_repo", "/root/.axon_site/_ro/trn_rl_repo"):
    if _p not in sys.path and os.path.isdir(_p):
        sys.path.append(_p)

import numpy as np

import concourse.bass as bass
import concourse.bacc as bacc
import concourse.mybir as mybir
import bass_rust
from concourse.tile import TileContext
from concourse.tile_rust import add_dep_helper

F32 = mybir.dt.float32
F16 = mybir.dt.float16

C, H, W = 64, 128, 128
S = H * W          # 16384
CW, GL = 8, 8      # weight channels, share planes
HB = 16            # row blocks
RB = H // HB       # rows per block = 8
XROWS = RB + 2     # 10 rows incl halo
XGL = XROWS * W    # 1280 elements per gl block in x_sb
SB = RB * W        # 1024 spatial elems per partition per gl


def _ap(base, dims, extra_offset=0):
    """Copy AP `base`, replace its [step,count] dims, bump offset.

    dims[0] is the partition dim: step "P" substitutes the base AP's own
    partition stride (flat element space, = free width).
    """
    c = base.copy()
    pstep = base.ap[0][0]
    dims = [[pstep if s == "P" else s, n] for s, n in dims]
    c.ap = bass_rust.VecI64Pair(dims)
    if extra_offset:
        c.offset = c.offset + extra_offset
    return c


def build_program():
    nc = bacc.Bacc("TRN2", target_bir_lowering=False, debug=False)
    x_d = nc.dram_tensor("x", [C, S], F32, kind="ExternalInput")
    w_d = nc.dram_tensor("w", [CW, 9, S], F32, kind="ExternalInput")
    o_d = nc.dram_tensor("out", [C, S], F32, kind="ExternalOutput")

    with TileContext(nc) as tc:
        with tc.tile_pool(name="main", bufs=1) as pool, \
             tc.tile_pool(name="qtree", bufs=2) as qpool, \
             tc.tile_pool(name="os", bufs=2) as opool, \
             tc.tile_pool(name="o32", bufs=2) as o32pool:
            # fp16 working set, filled by gpsimd SWDGE cast-DMAs (f32 in
            # DRAM -> fp16 in SBUF, converted in the DMA datapath).  No
            # f32 staging tiles, no engine-side input casts, and the
            # SWDGE path is not gated by the ~10us HWDGE preamble.
            x16 = pool.tile([128, GL * XGL + 4], F16)
            w16 = pool.tile([128, 9 * SB], F16)

            # zero the vertical halo rows that have no source data:
            # r=0 at hb=0 (partitions 0..8), r=9 at hb=15 (partitions
            # 120..128); the in-range partitions are overwritten by DMA.
            nc.vector.memset(
                _ap(x16[:], [["P", 128], [1, 2]]), 0.0)
            nc.vector.memset(
                _ap(x16[:], [["P", 128], [1, 2]],
                    extra_offset=2 + GL * XGL), 0.0)
            nc.vector.memset(
                _ap(x16[:], [["P", 128], [XGL, GL], [1, W]],
                    extra_offset=2), 0.0)
            nc.vector.memset(
                _ap(x16[:], [["P", 128], [XGL, GL], [1, W]],
                    extra_offset=2 + (XROWS - 1) * W), 0.0)

            def load_w_k(k):
                nc.gpsimd.dma_start(
                    out=_ap(w16[:], [["P", 128], [1, SB]],
                            extra_offset=k * SB),
                    in_=_ap(w_d.ap(), [[SB, HB], [9 * S, CW], [1, SB]],
                            extra_offset=k * S))

            def load_x_main(gl):
                # partitions 8..120 (hb 1..14): rows hb*8-1 .. hb*8+9 = one
                # 1280-element contiguous run of channel c per partition.
                # (per-gl: the DMA AP balancer can't split a 4D source.)
                nc.gpsimd.dma_start(
                    out=_ap(x16[8:120], [["P", 112], [1, XGL]],
                            extra_offset=2 + gl * XGL),
                    in_=_ap(x_d.ap(), [[RB * W, HB - 2], [GL * S, CW],
                                       [1, XGL]],
                            extra_offset=gl * S + (RB - 1) * W))

            def load_x_edges4(g0):
                # hb=0 (partitions 0..8): rows r=1..9 = x rows 0..8
                nc.gpsimd.dma_start(
                    out=_ap(x16[0:8], [["P", 8], [XGL, 4],
                                       [1, (XROWS - 1) * W]],
                            extra_offset=2 + g0 * XGL + W),
                    in_=_ap(x_d.ap(), [[GL * S, CW], [S, 4],
                                       [1, (XROWS - 1) * W]],
                            extra_offset=g0 * S))
                # hb=15 (partitions 120..128): rows r=0..8 = x rows 119..127
                nc.gpsimd.dma_start(
                    out=_ap(x16[120:128], [["P", 8], [XGL, 4],
                                           [1, (XROWS - 1) * W]],
                            extra_offset=2 + g0 * XGL),
                    in_=_ap(x_d.ap(), [[GL * S, CW], [S, 4],
                                       [1, (XROWS - 1) * W]],
                            extra_offset=g0 * S + (H - XROWS + 1) * W))

            # Issue order (single gpsimd sequencer, ~1us of descriptor
            # generation per DMA): quad0's working set first, then w
            # planes ahead of quad1's x so the per-k chain never starves.
            load_w_k(0)
            for gl in range(4):
                load_x_main(gl)
            load_x_edges4(0)
            for k in range(1, 6):
                load_w_k(k)
            for gl in range(4, 8):
                load_x_main(gl)
            load_w_k(6)
            load_w_k(7)
            load_w_k(8)
            load_x_edges4(4)

            # ---- compute (all DVE) ----
            # tap (dh, dw): out[h', w] += w_k[h', w] * x[r=h'+dh, w+dw-1];
            # dw=0 skips output col 0, dw=2 skips output col W-1 (their
            # x operand is the zero pad).
            def out_dma(gl, src):
                # sync queue is idle once the w/x0 loads drain (~20us);
                # keeping stores off scalar speeds x2..7 delivery.
                nc.sync.dma_start(
                    out=_ap(o_d.ap(), [[RB * W, HB], [GL * S, CW], [1, SB]],
                            extra_offset=gl * S),
                    in_=src)

            prev_last = None   # pin gl order: the static scheduler
            # otherwise reorders chains by its (wrong) DMA timing model,
            # head-of-line blocking the DVE queue on late inputs.

            # All gls use mult+add chains: 2 DVE ops per arriving w
            # plane matches the DMA delivery rate during the ramp (wide
            # tree reductions and full-width taps gated on w-border
            # memsets both measured slower end-to-end).
            # tap (dh, dw): out[h', w] += w_k[h', w] * x[r=h'+dh, w+dw-1];
            # dw=0 skips output col 0, dw=2 skips col W-1 (their x operand
            # is the zero pad, so the contribution is zero).
            # Process gl QUADS per DVE op (4D APs, w broadcast across the
            # quad via stride 0): quarters the DVE instruction count, so
            # the ~170ns/op fixed overhead is amortized over 4x the
            # elements.
            NG = 4                     # gls per DVE op
            for g0 in range(0, GL, NG):
                acc_t = opool.tile([128, NG * SB], F16, tag="o",
                                   name="acc_t")
                o32 = o32pool.tile([128, NG * SB], F32, tag="o32",
                                   name="o32")
                last_pair = g0 + NG >= GL
                first = True
                for k in range(9):
                    dh, dw = divmod(k, 3)
                    w0 = 1 if dw == 0 else 0          # output start col
                    cnt = W - 1 if dw != 1 else W     # output width
                    xoff = 2 + g0 * XGL + dh * W + (1 if dw == 2 else 0)
                    xvw = _ap(x16[:], [["P", 128], [XGL, NG], [W, RB],
                                       [1, cnt]],
                              extra_offset=xoff)
                    wvw = _ap(w16[:], [["P", 128], [0, NG], [W, RB],
                                       [1, cnt]],
                              extra_offset=k * SB + w0)
                    accw = _ap(acc_t[:], [["P", 128], [SB, NG], [W, RB],
                                          [1, cnt]],
                               extra_offset=w0)
                    if first:
                        # k=0 (a dw=0 tap) leaves output col 0 unwritten:
                        # zero the two border columns once.
                        nc.vector.memset(
                            _ap(acc_t[:], [["P", 128], [SB, NG], [W, RB],
                                           [W - 1, 2]]), 0.0)
                        m = nc.vector.tensor_mul(out=accw, in0=xvw,
                                                 in1=wvw)
                        if prev_last is not None:
                            add_dep_helper(m.ins, prev_last.ins, sync=False,
                                           reason="gl chain order")
                        first = False
                    else:
                        tmp = qpool.tile([128, NG * SB], F16, tag="tmp",
                                         name="tmp")
                        t = _ap(tmp[:], [["P", 128], [SB, NG], [W, RB],
                                         [1, cnt]],
                                extra_offset=w0)
                        nc.vector.tensor_mul(out=t, in0=xvw, in1=wvw)
                        if last_pair and k == 8:
                            # final add writes f32 directly: runs 1x but
                            # skips the serial ACT out-cast on the tail.
                            prev_last = nc.vector.tensor_add(
                                out=_ap(o32[:], [["P", 128], [SB, NG],
                                                 [W, RB], [1, cnt]],
                                        extra_offset=w0),
                                in0=accw, in1=t)
                            # col W-1 of o32: k=8 (dw=2) never writes it
                            # and its contribution there is zero, so it
                            # comes straight from acc_t (k<=7 sum).
                            nc.vector.tensor_scalar_add(
                                out=_ap(o32[:], [["P", 128], [SB, NG],
                                                 [W, RB]],
                                        extra_offset=W - 1),
                                in0=_ap(acc_t[:], [["P", 128], [SB, NG],
                                                   [W, RB]],
                                        extra_offset=W - 1),
                                scalar1=0.0)
                        else:
                            prev_last = nc.vector.tensor_add(out=accw,
                                                             in0=accw,
                                                             in1=t)
                if not last_pair:
                    # fp16 acc -> f32 staging on ACT, then store
                    nc.scalar.copy(
                        out=_ap(o32[:], [["P", 128], [1, NG * SB]]),
                        in_=_ap(acc_t[:], [["P", 128], [1, NG * SB]]))
                for g in range(NG):
                    out_dma(g0 + g, _ap(o32[:], [["P", 128], [1, SB]],
                                        extra_offset=g * SB))

    nc.compile()
    return nc


_NC_CACHE = None


def _get_nc():
    global _NC_CACHE
    if _NC_CACHE is None:
        _NC_CACHE = build_program()
    return _NC_CACHE


def kernel(input, weight):
    """input: [8,64,128,128] f32, weight: [8,8,9,16384] f32 ->
    [8,64,128,128] f32."""
    from concourse.bass_utils import run_bass_kernel_spmd

    x = np.ascontiguousarray(np.asarray(input, dtype=np.float32))
    w = np.ascontiguousarray(np.asarray(weight, dtype=np.float32))
    N = x.shape[0]
    nc = _get_nc()
    in_maps = [{"x": x[i].reshape(C, S), "w": w[i].reshape(CW, 9, S)}
               for i in range(N)]
    res = run_bass_kernel_spmd(nc, in_maps, core_ids=list(range(N)))
    out = np.stack([res.results[i]["out"].reshape(C, H, W) for i in range(N)])
    return out

